# revision 27
# baseline (speedup 1.0000x reference)
"""Fused cross-attention kernel for Trainium2, data-parallel over batch on 8 cores.

Uses the low-rank structure of cross-attention (L=77 << D=512) plus token-mask
compaction. The text-side factors are folded on the host:
  tn  = LayerNorm(text)                 (exact, incl. beta)
  keep only unmasked tokens (count <= Lc = max over batch), then
  W2  = [Wq_h @ K_h^T]_h   [512, H*Lc]  (K = (tn @ Wk)[kept])
  W3  = [V_h @ Wo_h]_h     [H*Lc, 512]  (V = (tn @ Wv)[kept])
with zero padding up to Lc per head; padded score cols give exp(0)=1 which the
denominator correction (negcnt = count - Lc per head) removes exactly, and
padded W3 rows are zero so they add nothing to the output.

Per core (one batch element) the device runs, per 128-query tile:
  S    = X @ W2              [128, HL]   (2 fp8 DoubleRow MMs, contract 256 ea)
  E    = exp(S * scale)                  (no max-sub: |S*scale| < 3)
  A    = E / (rowsum_h(E) + negcnt_h)    (masked-softmax via count fix)
  A^T  = PE-transpose in ceil(HL/128) chunks
  out  = sum_c A^T_c.T @ W3_c [128, 512] (bf16 accumulating MMs)
"""

import math
import sys

sys.path.insert(0, "/opt/trn_rl_repo")

import numpy as np
import ml_dtypes

import concourse.bass as bass
import concourse.mybir as mybir
import concourse.tile as tile
from concourse import bacc
from concourse.bass_utils import run_bass_kernel_spmd
from concourse.masks import make_identity

N_CORES = 8
B, T, S_, D, L, H = 8, 64, 196, 512, 77, 4
DH = D // H  # 128
NQ = T * S_  # 12544
LN_EPS = 1e-6
SCALE = float(DH) ** -0.5
P = 128
NCH = D // P  # 4 chunks of the feature dim

F32 = mybir.dt.float32
BF16 = mybir.dt.bfloat16
F8E4 = mybir.dt.float8e4

# fp8 (e4m3) scores path: X and W2 in fp8, scores via DoubleRow matmuls.
# Off: with mask compaction the scores matmul is LDWEIGHTS-bound, so fp8
# DoubleRow no longer pays for its numerics cost; worse, the smaller PE load
# lets the PE micro-idle and HAM-throttle to 1.2GHz (measured 181us vs 164).
FP8_SCORES = False
W2SCALE = 8.0  # host multiplies W2 by this pre-fp8-cast; exp scale divides it out

LAST_RESULTS = None  # BassKernelResults of the most recent run (for test harness)
_PROGRAM_CACHE = {}


def build_program(nq=NQ, lc=41):
    """One SPMD program; all 8 cores run it on their own batch element."""
    hl = H * lc
    ncc = (hl + P - 1) // P  # chunks of the compacted (head, token) dim
    w2w = (hl + 15) // 16 * 16 if FP8_SCORES else hl  # 16B-aligned k-tile stride

    nc = bacc.Bacc("TRN2", target_bir_lowering=False, debug=False, num_devices=N_CORES)

    xdt = F8E4 if FP8_SCORES else BF16
    xt = nc.dram_tensor("xt", [D, nq], xdt, kind="ExternalInput").ap()
    w2 = nc.dram_tensor("w2", [D, w2w], xdt, kind="ExternalInput").ap()
    w3 = nc.dram_tensor("w3", [ncc * P, D], BF16, kind="ExternalInput").ap()
    negcnt = nc.dram_tensor("negcnt", [P, 1], F32, kind="ExternalInput").ap()
    out = nc.dram_tensor("out", [nq, D], BF16, kind="ExternalOutput").ap()

    ntiles = nq // P  # 98
    # Ramped group sizes so PE never starves while the DMA pipe fills
    groups = []
    t0 = 0
    for gt in (2, 3, 4):
        groups.append((t0, gt))
        t0 += gt
    GT = 7
    while t0 < ntiles:
        gt = min(GT, ntiles - t0)
        groups.append((t0, gt))
        t0 += gt

    with tile.TileContext(nc) as tc:
        with (
            tc.tile_pool(name="const", bufs=1) as const,
            tc.tile_pool(name="xtp", bufs=3) as xtp,
            tc.tile_pool(name="attp", bufs=6) as attp,
            tc.tile_pool(name="smalls", bufs=24) as smalls,
            tc.tile_pool(name="outp", bufs=4) as outp,
            tc.tile_pool(name="ps_sc", bufs=3, space="PSUM") as ps_sc,
            tc.tile_pool(name="ps_at", bufs=2, space="PSUM") as ps_at,
            tc.tile_pool(name="ps_out", bufs=3, space="PSUM") as ps_out,
        ):
            # ---- prolog loads; w2 + first xt group gate the first matmul ----
            w2_sb = const.tile([P, NCH, w2w], xdt, tag="w2")
            nc.sync.dma_start(out=w2_sb[:], in_=w2.rearrange("(c p) n -> p c n", p=P))

            xt_r = xt.rearrange("(c p) q -> p c q", p=P)
            xt_sbs = {}
            g0, gt0 = groups[0]
            xt_sbs[g0] = xtp.tile([P, NCH, gt0 * P], xdt, tag="xt", name="xt_sb")
            nc.sync.dma_start(out=xt_sbs[g0][:], in_=xt_r[:, :, : gt0 * P])

            w3_sb = const.tile([P, ncc, D], BF16, tag="w3")
            nc.scalar.dma_start(out=w3_sb[:], in_=w3.rearrange("(c p) n -> p c n", p=P))
            negcnt_sb = const.tile([P, 1], F32, tag="negcnt")
            nc.scalar.dma_start(out=negcnt_sb[:], in_=negcnt)

            ident = const.tile([P, P], BF16)
            make_identity(nc, ident)

            # PE warm-up: dummy transposes keep the PE busy through the DMA
            # head so the HAM clock-gate reaches 8/8 before real matmuls start.
            for _ in range(40):
                warm = ps_at.tile([P, ncc * P], BF16, tag="ps_a", name="warm")
                nc.tensor.transpose(warm[:, :P], ident[:], ident[:])

            # ---- main loop, software-pipelined ----
            # The PE executes matmuls strictly in program order, so the score
            # matmuls are emitted LA tiles ahead of the same tile's transposes:
            # by the time the PE reaches transpose(j), the ACT/DVE/GpSimd
            # softmax chain for tile j has had LA tile-periods to finish.
            # The PSUM->SBUF output copy trails one further tile so the ACT
            # FIFO never makes exp(i) wait on an out-copy tied to recent PE
            # work.
            LA = 3
            group_of = {}
            for t0, gt in groups:
                for t in range(t0, t0 + gt):
                    group_of[t] = (t0, gt)

            st = {}  # per-tile in-flight tiles
            for i in range(ntiles + LA + 1):
                if i < ntiles:
                    g0, gt = group_of[i]
                    if g0 not in xt_sbs:
                        xt_sb = xtp.tile([P, NCH, gt * P], xdt, tag="xt")
                        nc.sync.dma_start(
                            out=xt_sb[:], in_=xt_r[:, :, g0 * P : (g0 + gt) * P]
                        )
                        xt_sbs[g0] = xt_sb
                    xt_sb = xt_sbs[g0]
                    tq = slice((i - g0) * P, (i - g0 + 1) * P)
                    ps_s = ps_sc.tile([P, hl], F32, tag="ps_s")
                    if FP8_SCORES:
                        for c in range(2):
                            nc.tensor.matmul(
                                ps_s[:],
                                xt_sb[:, 2 * c : 2 * c + 2, tq],
                                w2_sb[:, 2 * c : 2 * c + 2, :hl],
                                start=(c == 0),
                                stop=(c == 1),
                                perf_mode=mybir.MatmulPerfMode.DoubleRow,
                            )
                    else:
                        for kc in range(NCH):
                            nc.tensor.matmul(
                                ps_s[:],
                                xt_sb[:, kc, tq],
                                w2_sb[:, kc, :],
                                start=(kc == 0),
                                stop=(kc == NCH - 1),
                            )
                    exp_sb = attp.tile([P, hl], BF16, tag="exp")
                    nc.scalar.activation(
                        exp_sb[:], ps_s[:], mybir.ActivationFunctionType.Exp,
                        scale=SCALE / W2SCALE if FP8_SCORES else SCALE,
                    )
                    sumexp = smalls.tile([P, H], F32, tag="sumexp")
                    nc.vector.reduce_sum(
                        out=sumexp[:],
                        in_=exp_sb[:].rearrange("p (h l) -> p h l", h=H),
                        axis=mybir.AxisListType.X,
                    )
                    sumadj = smalls.tile([P, H], F32, tag="sumadj")
                    nc.gpsimd.tensor_scalar_add(sumadj[:], sumexp[:], negcnt_sb[:])
                    recip = smalls.tile([P, H], F32, tag="recip")
                    nc.vector.reciprocal_approx_fast(recip[:], sumadj[:])
                    attn_sb = attp.tile([P, hl], BF16, tag="attn")
                    nc.gpsimd.tensor_mul(
                        attn_sb[:].rearrange("p (h l) -> p h l", h=H),
                        exp_sb[:].rearrange("p (h l) -> p h l", h=H),
                        recip[:].to_broadcast([P, H, lc]),
                    )
                    st[i] = {"attn": attn_sb}

                j = i - LA
                if 0 <= j < ntiles:
                    attn_sb = st[j].pop("attn")
                    ps_a = ps_at.tile([P, ncc * P], BF16, tag="ps_a")
                    for c in range(ncc):
                        cw = min(P, hl - c * P)
                        nc.tensor.transpose(
                            ps_a[:cw, c * P : (c + 1) * P],
                            attn_sb[:, c * P : c * P + cw],
                            ident[:],
                        )
                    attnT_sb = attp.tile([P, ncc, P], BF16, tag="attnT")
                    for c in range(ncc):
                        cw = min(P, hl - c * P)
                        nc.vector.tensor_copy(
                            attnT_sb[:cw, c, :], ps_a[:cw, c * P : (c + 1) * P]
                        )
                    ps_o = ps_out.tile([P, D], F32, tag="ps_o")
                    for c in range(ncc):
                        cw = min(P, hl - c * P)
                        nc.tensor.matmul(
                            ps_o[:],
                            attnT_sb[:cw, c, :],
                            w3_sb[:cw, c, :],
                            start=(c == 0),
                            stop=(c == ncc - 1),
                        )
                    st[j]["ps_o"] = ps_o

                k = i - LA - 1
                if 0 <= k < ntiles:
                    ps_o = st.pop(k)["ps_o"]
                    out_sb = outp.tile([P, D], BF16, tag="out")
                    nc.scalar.copy(out_sb[:], ps_o[:])
                    nc.sync.dma_start(out=out[k * P : (k + 1) * P, :], in_=out_sb[:])

    nc.compile()
    return nc


def _get_program(nq=NQ, lc=41):
    key = (nq, lc)
    if key not in _PROGRAM_CACHE:
        _PROGRAM_CACHE[key] = build_program(nq, lc)
    return _PROGRAM_CACHE[key]


def prep_core_inputs(visual_feat, text_feat, token_mask, wq, wk, wv, wo,
                     ln_gamma, ln_beta):
    """Host-side prep: shard over batch, compact masked tokens, fold the text
    side into W2/W3."""
    vf = np.ascontiguousarray(visual_feat.reshape(B, -1, D))

    # Exact LayerNorm (f32, biased variance, incl. beta)
    mu = text_feat.mean(-1, keepdims=True)
    var = np.square(text_feat - mu).mean(-1, keepdims=True)
    tn = (text_feat - mu) / np.sqrt(var + LN_EPS) * ln_gamma + ln_beta  # [B, L, D]

    mask = np.asarray(token_mask).astype(bool)  # [B, L]
    counts = mask.sum(1)
    lc = int(counts.max())
    hl = H * lc
    ncc = (hl + P - 1) // P
    w2w = (hl + 15) // 16 * 16 if FP8_SCORES else hl

    k_all = tn @ wk  # [B, L, D]
    v_all = tn @ wv
    wq4 = wq.reshape(D, H, DH)
    wo4 = wo.reshape(H, DH, D)

    in_maps = []
    for b in range(B):
        cnt = int(counts[b])
        kc = k_all[b][mask[b]].reshape(cnt, H, DH)  # [cnt, H, DH]
        vc = v_all[b][mask[b]].reshape(cnt, H, DH)
        # W2[d, h*lc+l] = sum_e Wq[d,(h,e)] K[l,(h,e)]
        w2_b = np.zeros((D, H, lc), np.float32)
        w2_b[:, :, :cnt] = np.einsum("dhe,lhe->dhl", wq4, kc, optimize=True)
        # W3[h*lc+l, d] = sum_e V[l,(h,e)] Wo[(h,e),d]
        w3_b = np.zeros((H, lc, D), np.float32)
        w3_b[:, :cnt, :] = np.einsum("lhe,hed->hld", vc, wo4, optimize=True)
        w3_pad = np.zeros((ncc * P, D), np.float32)
        w3_pad[:hl] = w3_b.reshape(hl, D)

        xt_b = np.ascontiguousarray(vf[b].T)
        if FP8_SCORES:
            # TRN FP8_EXP4 is e4m3 with max +-240 (256+ decodes as inf/nan)
            xt_c = np.clip(xt_b, -240, 240).astype(ml_dtypes.float8_e4m3fn)
            w2_c = np.zeros((D, w2w), np.float32)
            w2_c[:, :hl] = w2_b.reshape(D, hl) * W2SCALE
            w2_c = np.clip(w2_c, -240, 240).astype(ml_dtypes.float8_e4m3fn)
        else:
            xt_c = xt_b.astype(ml_dtypes.bfloat16)
            w2_c = w2_b.reshape(D, hl).astype(ml_dtypes.bfloat16)
        negcnt_b = np.full((P, 1), float(cnt - lc), np.float32)
        in_maps.append({
            "xt": xt_c,
            "w2": w2_c,
            "w3": w3_pad.astype(ml_dtypes.bfloat16),
            "negcnt": negcnt_b,
        })
    return in_maps, lc


def kernel(visual_feat, text_feat, token_mask, Wq, Wk, Wv, Wo, ln_gamma, ln_beta):
    global LAST_RESULTS
    visual_feat = np.asarray(visual_feat, np.float32)
    text_feat = np.asarray(text_feat, np.float32)
    token_mask = np.asarray(token_mask)

    in_maps, lc = prep_core_inputs(
        visual_feat, text_feat, token_mask,
        np.asarray(Wq, np.float32), np.asarray(Wk, np.float32),
        np.asarray(Wv, np.float32), np.asarray(Wo, np.float32),
        np.asarray(ln_gamma, np.float32), np.asarray(ln_beta, np.float32),
    )
    nc = _get_program(NQ, lc)
    res = run_bass_kernel_spmd(nc, in_maps, core_ids=list(range(N_CORES)))
    LAST_RESULTS = res
    out = np.stack([res.results[b]["out"].astype(np.float32) for b in range(B)], axis=0)
    return out.reshape(B, T, S_, D)
